# revision 4
# baseline (speedup 1.0000x reference)
"""Trainium2 Bass kernel for nn_AttnBlock (B=8, C=64, H=W=64).

Data-parallel: 1 batch per NeuronCore (8 cores). Per core, full
flash-style attention over N=4096 positions with C=64 channels,
never materializing the (N, N) score tensor in HBM.

Per-core pipeline (all on-chip, x kept resident in SBUF):
  1. GroupNorm(32 groups of 2 channels) via bn_stats + pair-combine matmul.
  2. Transpose xn (c, n) -> XT (n, c) tiles via PE transposes.
  3. Width-axis linear projections q/k/v using block-diagonal weights:
       q, k in (c, n) layout; v in (n, c) layout with a ones column
       appended (row sums of exp(S) fall out of the AV matmul for free).
  4. For each 512-wide chunk of query positions:
       S^T tiles = K_mtile^T-style matmuls (fp32r, full PE rate),
       P = exp(S * C^-0.5) on the scalar engine straight out of PSUM,
       attn_out[c, n] (+ row-sum l[n]) accumulated via AV matmuls,
       epilogue: transpose, normalize by 1/l, P-projection, +bias,
       transpose back, +residual, DMA out.

Self-contained: hardcodes all shapes; no file reads.
"""

import numpy as np
from contextlib import ExitStack

import concourse.bass as bass
import concourse.bacc as bacc
import concourse.tile as tile
from concourse import mybir

F32 = mybir.dt.float32
R32 = mybir.dt.float32r
BF16 = mybir.dt.bfloat16

C = 64
N = 4096          # H*W
NCH = 8           # n-chunks of 512
CHW = 512         # chunk width
MT = 32           # m-tiles of 128
EPS = 1e-5
SCALE = 1.0 / 8.0  # C ** -0.5

# m-tile groups per exp-activation (3 psum banks per s-tile, double buffered)
GROUPS = [(m0, min(3, MT - m0)) for m0 in range(0, MT, 3)]

INPUT_NAMES = [
    "x", "gamma", "beta", "p2", "wqbd", "wkbd", "wvbd", "wpbd",
    "bq2d", "bk2d", "bv2", "bp2", "ident",
]


def attn_body(ctx: ExitStack, tc: "tile.TileContext", ins: dict, y_d):
    nc = tc.nc
    Exp = mybir.ActivationFunctionType.Exp
    Sqrt = mybir.ActivationFunctionType.Sqrt
    mult = mybir.AluOpType.mult
    add = mybir.AluOpType.add
    sub = mybir.AluOpType.subtract

    persist = ctx.enter_context(tc.tile_pool(name="persist", bufs=1))
    sm = ctx.enter_context(tc.tile_pool(name="sm", bufs=2))

    # ---- persistent SBUF tiles ----
    X = persist.tile([C, N], F32, tag="X")
    XN = persist.tile([C, N], F32, tag="XN")
    XT = persist.tile([128, 2048], F32, tag="XT")      # ((h,w), c) chunks
    Q = persist.tile([C, N], BF16, tag="Q")
    K = persist.tile([C, N], BF16, tag="K")
    VT1 = persist.tile([128, MT, 65], BF16, tag="VT1")  # ((H,j), c | 1)
    OUT = persist.tile([C, N], F32, tag="OUT")

    GM = persist.tile([C, 1], F32, tag="GM")
    BT = persist.tile([C, 1], F32, tag="BT")
    P2 = persist.tile([C, C], F32, tag="P2")
    WQ = persist.tile([128, 128], F32, tag="WQ")
    WK = persist.tile([128, 128], F32, tag="WK")
    WV = persist.tile([128, 128], F32, tag="WV")
    WP = persist.tile([128, 128], F32, tag="WP")
    BQ2D = persist.tile([C, 128], F32, tag="BQ2D")
    BK2D = persist.tile([C, 128], F32, tag="BK2D")
    BV2 = persist.tile([128, 1], F32, tag="BV2")
    BP2 = persist.tile([128, 1], F32, tag="BP2")
    ID = persist.tile([128, 128], F32, tag="ID")
    EPS_T = persist.tile([C, 1], F32, tag="EPS_T")
    ZB = persist.tile([128, 1], F32, tag="ZB")

    # ---- DMA inputs ----
    nc.sync.dma_start(out=X, in_=ins["x"])
    nc.sync.dma_start(out=GM, in_=ins["gamma"])
    nc.sync.dma_start(out=BT, in_=ins["beta"])
    nc.sync.dma_start(out=P2, in_=ins["p2"])
    nc.sync.dma_start(out=WQ, in_=ins["wqbd"])
    nc.sync.dma_start(out=WK, in_=ins["wkbd"])
    nc.sync.dma_start(out=WV, in_=ins["wvbd"])
    nc.sync.dma_start(out=WP, in_=ins["wpbd"])
    nc.sync.dma_start(out=BQ2D, in_=ins["bq2d"])
    nc.sync.dma_start(out=BK2D, in_=ins["bk2d"])
    nc.sync.dma_start(out=BV2, in_=ins["bv2"])
    nc.sync.dma_start(out=BP2, in_=ins["bp2"])
    nc.sync.dma_start(out=ID, in_=ins["ident"])
    nc.vector.memset(EPS_T, EPS)
    nc.vector.memset(ZB, 0.0)
    nc.vector.memset(VT1[:, :, 64:65], 1.0)
    tc.strict_bb_all_engine_barrier()

    # ---- GroupNorm ----
    with tc.tile_pool(name="pst", space="PSUM", bufs=2) as pst:
        stats = sm.tile([C, 8, 6], F32, tag="stats")
        xg = X.rearrange("p (s f) -> p s f", s=8)
        for s in range(8):
            nc.vector.bn_stats(out=stats[:, s, :], in_=xg[:, s, :])
        mv = sm.tile([C, 2], F32, tag="mv")
        nc.vector.bn_aggr(out=mv, in_=stats)

        # st = [mean, E[x^2]] per channel
        st = sm.tile([C, 2], F32, tag="st")
        nc.vector.tensor_copy(out=st[:, 0:1], in_=mv[:, 0:1])
        msq = sm.tile([C, 1], F32, tag="msq")
        nc.vector.tensor_tensor(out=msq, in0=mv[:, 0:1], in1=mv[:, 0:1], op=mult)
        nc.vector.tensor_tensor(out=st[:, 1:2], in0=msq, in1=mv[:, 1:2], op=add)

        # group (pair) averages, broadcast back to both partitions
        mg_ps = pst.tile([C, 2], F32, tag="small")
        nc.tensor.matmul(mg_ps, lhsT=P2, rhs=st, start=True, stop=True)

        mu = sm.tile([C, 1], F32, tag="mu")
        nc.vector.tensor_copy(out=mu, in_=mg_ps[:, 0:1])
        musq = sm.tile([C, 1], F32, tag="musq")
        nc.vector.tensor_tensor(out=musq, in0=mu, in1=mu, op=mult)
        ve = sm.tile([C, 1], F32, tag="ve")
        nc.vector.tensor_tensor(out=ve, in0=mg_ps[:, 1:2], in1=musq, op=sub)
        # ve = var_g; add eps
        nc.vector.tensor_scalar_add(out=ve, in0=ve, scalar1=EPS)

        sq = sm.tile([C, 1], F32, tag="sq")
        nc.scalar.activation(out=sq, in_=ve, func=Sqrt, bias=ZB[0:C, :], scale=1.0)
        r0 = sm.tile([C, 1], F32, tag="r0")
        nc.vector.reciprocal(out=r0, in_=sq)
        # one Newton step for rsqrt: y1 = r0 * (1.5 - 0.5 * ve * r0^2)
        t0 = sm.tile([C, 1], F32, tag="t0")
        nc.vector.tensor_tensor(out=t0, in0=r0, in1=r0, op=mult)
        nc.vector.tensor_tensor(out=t0, in0=t0, in1=ve, op=mult)
        nc.vector.tensor_scalar(out=t0, in0=t0, scalar1=-0.5, scalar2=1.5,
                                op0=mult, op1=add)
        rstd = sm.tile([C, 1], F32, tag="rstd")
        nc.vector.tensor_tensor(out=rstd, in0=r0, in1=t0, op=mult)

        sc = sm.tile([C, 1], F32, tag="sc")
        nc.vector.tensor_tensor(out=sc, in0=rstd, in1=GM, op=mult)
        t3 = sm.tile([C, 1], F32, tag="t3")
        nc.vector.tensor_tensor(out=t3, in0=mu, in1=sc, op=mult)
        sh = sm.tile([C, 1], F32, tag="sh")
        nc.vector.tensor_tensor(out=sh, in0=BT, in1=t3, op=sub)
        nc.vector.tensor_scalar(out=XN, in0=X, scalar1=sc, scalar2=sh,
                                op0=mult, op1=add)

        # ---- transpose xn -> XT ((h,w), c) ----
        for i in range(MT):
            tp = pst.tile([128, C], F32, tag="tp")
            nc.tensor.transpose(out=tp, in_=XN[:, i * 128:(i + 1) * 128],
                                identity=ID[0:C, 0:C])
            nc.vector.tensor_copy(out=XT[:, i * C:(i + 1) * C], in_=tp)

        # ---- projections ----
        for i in range(MT):
            qp = pst.tile([C, 128], F32, tag="qp")
            nc.tensor.matmul(qp, lhsT=XT[:, i * C:(i + 1) * C], rhs=WQ,
                             start=True, stop=True)
            nc.vector.tensor_tensor(out=Q[:, i * 128:(i + 1) * 128],
                                    in0=qp, in1=BQ2D, op=add)
        for i in range(MT):
            kp = pst.tile([C, 128], F32, tag="qp")
            nc.tensor.matmul(kp, lhsT=XT[:, i * C:(i + 1) * C], rhs=WK,
                             start=True, stop=True)
            nc.vector.tensor_tensor(out=K[:, i * 128:(i + 1) * 128],
                                    in0=kp, in1=BK2D, op=add)
        for i in range(MT):
            vp = pst.tile([128, C], F32, tag="tp")
            nc.tensor.matmul(vp, lhsT=WV, rhs=XT[:, i * C:(i + 1) * C],
                             start=True, stop=True)
            nc.vector.tensor_scalar_add(out=VT1[:, i, 0:C], in0=vp, scalar1=BV2)

    tc.strict_bb_all_engine_barrier()

    # ---- attention ----
    spool = ctx.enter_context(tc.tile_pool(name="spool", space="PSUM", bufs=2))
    opool = ctx.enter_context(tc.tile_pool(name="opool", space="PSUM", bufs=1))
    epool = ctx.enter_context(tc.tile_pool(name="epool", space="PSUM", bufs=1))
    ptpool = ctx.enter_context(tc.tile_pool(name="ptpool", bufs=3))
    esb = ctx.enter_context(tc.tile_pool(name="esb", bufs=2))

    for ch in range(NCH):
        nsl = slice(ch * CHW, (ch + 1) * CHW)
        po = opool.tile([128, CHW], F32, tag="po")
        for (m0, gsz) in GROUPS:
            ps = spool.tile([128, 3 * CHW], F32, tag="ps")
            for t in range(gsz):
                m = m0 + t
                nc.tensor.matmul(
                    ps[:, t * CHW:(t + 1) * CHW],
                    lhsT=K[:, m * 128:(m + 1) * 128],
                    rhs=Q[:, nsl],
                    start=True, stop=True)
            pt = ptpool.tile([128, 3 * CHW], BF16, tag="pt")
            nc.scalar.activation(out=pt[:, 0:gsz * CHW], in_=ps[:, 0:gsz * CHW],
                                 func=Exp, bias=ZB, scale=SCALE)
            for t in range(gsz):
                m = m0 + t
                nc.tensor.matmul(
                    po[0:65, :],
                    lhsT=VT1[:, m, :],
                    rhs=pt[:, t * CHW:(t + 1) * CHW],
                    start=(m == 0), stop=(m == MT - 1),
                    skip_group_check=True)

        # ---- epilogue: normalize, project, residual ----
        osb = esb.tile([65, CHW], F32, tag="osb")
        nc.vector.tensor_copy(out=osb, in_=po[0:65, :])
        for s4 in range(4):
            csl = slice(ch * CHW + s4 * 128, ch * CHW + (s4 + 1) * 128)
            pat = epool.tile([128, 65], F32, tag="ep")
            nc.tensor.transpose(out=pat, in_=osb[:, s4 * 128:(s4 + 1) * 128],
                                identity=ID[0:65, 0:65])
            rli = esb.tile([128, 1], F32, tag="rli")
            nc.vector.reciprocal(out=rli, in_=pat[:, 64:65])
            atn = esb.tile([128, C], F32, tag="atn")
            nc.vector.tensor_scalar_mul(out=atn, in0=pat[:, 0:C], scalar1=rli)
            pp = epool.tile([128, C], F32, tag="ep")
            nc.tensor.matmul(pp, lhsT=WP, rhs=atn, start=True, stop=True)
            otb = esb.tile([128, C], F32, tag="otb")
            nc.vector.tensor_scalar_add(out=otb, in0=pp, scalar1=BP2)
            pf = epool.tile([C, 128], F32, tag="ep")
            nc.tensor.transpose(out=pf, in_=otb, identity=ID)
            nc.vector.tensor_tensor(out=OUT[:, csl], in0=pf, in1=X[:, csl], op=add)
        nc.sync.dma_start(out=y_d[:, nsl], in_=OUT[:, nsl])


def build_nc():
    nc = bacc.Bacc("TRN2", target_bir_lowering=False, debug=False)
    shapes = {
        "x": [C, N], "gamma": [C, 1], "beta": [C, 1], "p2": [C, C],
        "wqbd": [128, 128], "wkbd": [128, 128], "wvbd": [128, 128],
        "wpbd": [128, 128], "bq2d": [C, 128], "bk2d": [C, 128],
        "bv2": [128, 1], "bp2": [128, 1], "ident": [128, 128],
    }
    ins = {k: nc.dram_tensor(k, v, F32, kind="ExternalInput").ap()
           for k, v in shapes.items()}
    y_d = nc.dram_tensor("y", [C, N], F32, kind="ExternalOutput").ap()
    with tile.TileContext(nc) as tc:
        with ExitStack() as ctx:
            attn_body(ctx, tc, ins, y_d)
    nc.compile()
    return nc


def host_params(inputs):
    """Build the small derived parameter arrays shared by all cores."""
    f = lambda k: np.asarray(inputs[k], np.float32)
    p = {}
    for nm, w in (("wqbd", "Wq"), ("wkbd", "Wk"), ("wvbd", "Wv"), ("wpbd", "Wp")):
        W = f(w)
        bd = np.zeros((128, 128), np.float32)
        bd[0:64, 0:64] = W.T
        bd[64:128, 64:128] = W.T
        p[nm] = bd
    p["gamma"] = f("gn_w").reshape(C, 1)
    p["beta"] = f("gn_b").reshape(C, 1)
    p["bq2d"] = np.broadcast_to(np.tile(f("bq"), 2)[None, :], (C, 128)).copy()
    p["bk2d"] = np.broadcast_to(np.tile(f("bk"), 2)[None, :], (C, 128)).copy()
    p["bv2"] = np.tile(f("bv"), 2).reshape(128, 1).astype(np.float32)
    p["bp2"] = np.tile(f("bp"), 2).reshape(128, 1).astype(np.float32)
    p["ident"] = np.eye(128, dtype=np.float32)
    p2 = np.zeros((C, C), np.float32)
    for g in range(C // 2):
        p2[2 * g:2 * g + 2, 2 * g:2 * g + 2] = 0.5
    p["p2"] = p2
    return p


_NC_CACHE = {}


def get_nc():
    if "nc" not in _NC_CACHE:
        _NC_CACHE["nc"] = build_nc()
    return _NC_CACHE["nc"]


def make_in_maps(inputs):
    x = np.asarray(inputs["x"], np.float32)
    B = x.shape[0]
    p = host_params(inputs)
    return [dict(p, x=np.ascontiguousarray(x[b].reshape(C, N))) for b in range(B)]


def kernel(**inputs):
    from concourse.bass_utils import run_bass_kernel_spmd
    x = np.asarray(inputs["x"], np.float32)
    B = x.shape[0]
    nc = get_nc()
    in_maps = make_in_maps(inputs)
    res = run_bass_kernel_spmd(nc, in_maps, core_ids=list(range(B)))
    y = np.stack([res.results[b]["y"].reshape(C, 64, 64) for b in range(B)])
    return y.astype(np.float32)


# revision 5
# speedup vs baseline: 1.2627x; 1.2627x over previous
"""Trainium2 Bass kernel for nn_AttnBlock (B=8, C=64, H=W=64).

Data-parallel: 1 batch per NeuronCore (8 cores). Per core, full
flash-style attention over N=4096 positions with C=64 channels,
never materializing the (N, N) score tensor in HBM.

Per-core pipeline (all on-chip, x kept resident in SBUF):
  1. GroupNorm(32 groups of 2 channels) via bn_stats + pair-combine matmul.
  2. Transpose xn (c, n) -> XT (n, c) bf16 tiles via PE transposes.
  3. Width-axis linear projections q/k/v (bf16) using block-diagonal
     weights: q, k in (c, n) layout; v in (n, c) layout with a ones
     column (row sums of exp(S) fall out of the AV matmul for free).
  4. For each 512-wide chunk of query positions:
       S^T tiles via bf16 matmuls (contraction over c),
       P = exp(S * C^-0.5) on the scalar engine straight out of PSUM,
       attn_out[c, n] (+ row-sum l[n]) accumulated via bf16 AV matmuls.
     The normalize/project/residual epilogue of chunk i is emitted in
     the middle of chunk i+1's main loop so the PE never starves.

Self-contained: hardcodes all shapes; no file reads.
"""

import numpy as np
from contextlib import ExitStack

import concourse.bass as bass
import concourse.bacc as bacc
import concourse.tile as tile
from concourse import mybir

F32 = mybir.dt.float32
BF16 = mybir.dt.bfloat16

C = 64
N = 4096          # H*W
NCH = 8           # n-chunks of 512
CHW = 512         # chunk width
MT = 32           # m-tiles of 128
EPS = 1e-5
SCALE = 1.0 / 8.0  # C ** -0.5

# m-tile groups per exp-activation (3 psum banks per s-tile, double buffered)
GROUPS = [(m0, min(3, MT - m0)) for m0 in range(0, MT, 3)]


def attn_body(ctx: ExitStack, tc: "tile.TileContext", ins: dict, y_d):
    nc = tc.nc
    Exp = mybir.ActivationFunctionType.Exp
    Sqrt = mybir.ActivationFunctionType.Sqrt
    mult = mybir.AluOpType.mult
    add = mybir.AluOpType.add
    sub = mybir.AluOpType.subtract

    persist = ctx.enter_context(tc.tile_pool(name="persist", bufs=1))
    sm = ctx.enter_context(tc.tile_pool(name="sm", bufs=2))

    # ---- persistent SBUF tiles ----
    X = persist.tile([C, N], F32, tag="X")
    XN = persist.tile([C, N], BF16, tag="XN")
    XT = persist.tile([128, 2048], BF16, tag="XT")      # ((h,w), c) chunks
    Q = persist.tile([C, N], BF16, tag="Q")
    K = persist.tile([C, N], BF16, tag="K")
    VT1 = persist.tile([128, MT, 65], BF16, tag="VT1")  # ((H,j), c | 1)
    OUT = persist.tile([C, N], F32, tag="OUT")

    GM = persist.tile([C, 1], F32, tag="GM")
    BT = persist.tile([C, 1], F32, tag="BT")
    P2 = persist.tile([C, C], F32, tag="P2")
    WQ = persist.tile([128, 128], BF16, tag="WQ")
    WK = persist.tile([128, 128], BF16, tag="WK")
    WV = persist.tile([128, 128], BF16, tag="WV")
    WP = persist.tile([128, 128], F32, tag="WP")
    BQ2D = persist.tile([C, 1024], F32, tag="BQ2D")
    BK2D = persist.tile([C, 1024], F32, tag="BK2D")
    BV2 = persist.tile([128, 1], F32, tag="BV2")
    BP2 = persist.tile([128, 1], F32, tag="BP2")
    ID = persist.tile([128, 128], F32, tag="ID")
    IDB = persist.tile([128, 128], BF16, tag="IDB")
    ZB = persist.tile([128, 1], F32, tag="ZB")

    # ---- DMA inputs (x split across queues) ----
    for i in range(4):
        nc.sync.dma_start(out=X[:, i * 1024:(i + 1) * 1024],
                          in_=ins["x"][:, i * 1024:(i + 1) * 1024])
    nc.sync.dma_start(out=GM, in_=ins["gamma"])
    nc.sync.dma_start(out=BT, in_=ins["beta"])
    nc.sync.dma_start(out=P2, in_=ins["p2"])
    nc.sync.dma_start(out=WQ, in_=ins["wqbd"])
    nc.sync.dma_start(out=WK, in_=ins["wkbd"])
    nc.sync.dma_start(out=WV, in_=ins["wvbd"])
    nc.sync.dma_start(out=WP, in_=ins["wpbd"])
    nc.sync.dma_start(out=BQ2D, in_=ins["bq2d"])
    nc.sync.dma_start(out=BK2D, in_=ins["bk2d"])
    nc.sync.dma_start(out=BV2, in_=ins["bv2"])
    nc.sync.dma_start(out=BP2, in_=ins["bp2"])
    nc.sync.dma_start(out=ID, in_=ins["ident"])
    nc.sync.dma_start(out=IDB, in_=ins["identb"])
    nc.vector.memset(ZB, 0.0)
    nc.vector.memset(VT1[:, :, 64:65], 1.0)
    tc.strict_bb_all_engine_barrier()

    # ---- GroupNorm stats ----
    with tc.tile_pool(name="pst", space="PSUM", bufs=2) as pst:
        stats = sm.tile([C, 8, 6], F32, tag="stats")
        xg = X.rearrange("p (s f) -> p s f", s=8)
        for s in range(8):
            nc.vector.bn_stats(out=stats[:, s, :], in_=xg[:, s, :])
        mv = sm.tile([C, 2], F32, tag="mv")
        nc.vector.bn_aggr(out=mv, in_=stats)

        # st = [mean, E[x^2]] per channel
        st = sm.tile([C, 2], F32, tag="st")
        nc.vector.tensor_copy(out=st[:, 0:1], in_=mv[:, 0:1])
        msq = sm.tile([C, 1], F32, tag="msq")
        nc.vector.tensor_tensor(out=msq, in0=mv[:, 0:1], in1=mv[:, 0:1], op=mult)
        nc.vector.tensor_tensor(out=st[:, 1:2], in0=msq, in1=mv[:, 1:2], op=add)

        # group (pair) averages, broadcast back to both partitions
        mg_ps = pst.tile([C, 2], F32, tag="tp4")
        nc.tensor.matmul(mg_ps, lhsT=P2, rhs=st, start=True, stop=True)

        mu = sm.tile([C, 1], F32, tag="mu")
        nc.vector.tensor_copy(out=mu, in_=mg_ps[:, 0:1])
        musq = sm.tile([C, 1], F32, tag="musq")
        nc.vector.tensor_tensor(out=musq, in0=mu, in1=mu, op=mult)
        ve = sm.tile([C, 1], F32, tag="ve")
        nc.vector.tensor_tensor(out=ve, in0=mg_ps[:, 1:2], in1=musq, op=sub)
        nc.vector.tensor_scalar_add(out=ve, in0=ve, scalar1=EPS)

        sq = sm.tile([C, 1], F32, tag="sq")
        nc.scalar.activation(out=sq, in_=ve, func=Sqrt, bias=ZB[0:C, :], scale=1.0)
        r0 = sm.tile([C, 1], F32, tag="r0")
        nc.vector.reciprocal(out=r0, in_=sq)
        # one Newton step for rsqrt: y1 = r0 * (1.5 - 0.5 * ve * r0^2)
        t0 = sm.tile([C, 1], F32, tag="t0")
        nc.vector.tensor_tensor(out=t0, in0=r0, in1=r0, op=mult)
        nc.vector.tensor_tensor(out=t0, in0=t0, in1=ve, op=mult)
        nc.vector.tensor_scalar(out=t0, in0=t0, scalar1=-0.5, scalar2=1.5,
                                op0=mult, op1=add)
        rstd = sm.tile([C, 1], F32, tag="rstd")
        nc.vector.tensor_tensor(out=rstd, in0=r0, in1=t0, op=mult)

        sc = sm.tile([C, 1], F32, tag="sc")
        nc.vector.tensor_tensor(out=sc, in0=rstd, in1=GM, op=mult)
        t3 = sm.tile([C, 1], F32, tag="t3")
        nc.vector.tensor_tensor(out=t3, in0=mu, in1=sc, op=mult)
        sh = sm.tile([C, 1], F32, tag="sh")
        nc.vector.tensor_tensor(out=sh, in0=BT, in1=t3, op=sub)

        # ---- per 1024-col block: normalize, transpose, project ----
        for blk in range(4):
            bsl = slice(blk * 1024, (blk + 1) * 1024)
            nc.vector.tensor_scalar(out=XN[:, bsl], in0=X[:, bsl],
                                    scalar1=sc, scalar2=sh, op0=mult, op1=add)
            # transpose 8 x (64,128) -> (128,64), batched 4 per psum tile
            for g in range(2):
                tp4 = pst.tile([128, 256], BF16, tag="tp4")
                for t in range(4):
                    i = blk * 8 + g * 4 + t
                    nc.tensor.transpose(out=tp4[:, t * 64:(t + 1) * 64],
                                        in_=XN[:, i * 128:(i + 1) * 128],
                                        identity=IDB[0:C, 0:C])
                i0 = blk * 8 + g * 4
                nc.vector.tensor_copy(out=XT[:, i0 * 64:(i0 + 4) * 64], in_=tp4)
            # q, k projections (8 matmuls -> one (64,1024) psum tile each)
            qp8 = pst.tile([C, 1024], F32, tag="qk")
            for t in range(8):
                i = blk * 8 + t
                nc.tensor.matmul(qp8[:, t * 128:(t + 1) * 128],
                                 lhsT=XT[:, i * C:(i + 1) * C], rhs=WQ,
                                 start=True, stop=True)
            nc.vector.tensor_tensor(out=Q[:, bsl], in0=qp8, in1=BQ2D, op=add)
            kp8 = pst.tile([C, 1024], F32, tag="qk")
            for t in range(8):
                i = blk * 8 + t
                nc.tensor.matmul(kp8[:, t * 128:(t + 1) * 128],
                                 lhsT=XT[:, i * C:(i + 1) * C], rhs=WK,
                                 start=True, stop=True)
            nc.vector.tensor_tensor(out=K[:, bsl], in0=kp8, in1=BK2D, op=add)
            # v projection (4 matmuls per (128,256) psum tile)
            for g in range(2):
                vp4 = pst.tile([128, 256], F32, tag="v4")
                for t in range(4):
                    i = blk * 8 + g * 4 + t
                    nc.tensor.matmul(vp4[:, t * 64:(t + 1) * 64],
                                     lhsT=WV, rhs=XT[:, i * C:(i + 1) * C],
                                     start=True, stop=True)
                i0 = blk * 8 + g * 4
                nc.vector.tensor_scalar_add(
                    out=VT1[:, i0:i0 + 4, 0:C],
                    in0=vp4.rearrange("p (a b) -> p a b", a=4),
                    scalar1=BV2)

    tc.strict_bb_all_engine_barrier()

    # ---- attention ----
    spool = ctx.enter_context(tc.tile_pool(name="spool", space="PSUM", bufs=2))
    opool = ctx.enter_context(tc.tile_pool(name="opool", space="PSUM", bufs=1))
    epool = ctx.enter_context(tc.tile_pool(name="epool", space="PSUM", bufs=1))
    ptpool = ctx.enter_context(tc.tile_pool(name="ptpool", bufs=3))
    esb = ctx.enter_context(tc.tile_pool(name="esb", bufs=2))

    osbs = {}

    def epilogue(ch):
        """Normalize by 1/l, project through Wp, add bias+residual, DMA out."""
        osb = osbs.pop(ch)
        nsl = slice(ch * CHW, (ch + 1) * CHW)
        for s4 in range(4):
            csl = slice(ch * CHW + s4 * 128, ch * CHW + (s4 + 1) * 128)
            pat = epool.tile([128, 65], F32, tag="ep")
            nc.tensor.transpose(out=pat, in_=osb[:, s4 * 128:(s4 + 1) * 128],
                                identity=ID[0:65, 0:65])
            rli = esb.tile([128, 1], F32, tag="rli")
            nc.vector.reciprocal(out=rli, in_=pat[:, 64:65])
            atn = esb.tile([128, C], F32, tag="atn")
            nc.vector.tensor_scalar_mul(out=atn, in0=pat[:, 0:C], scalar1=rli)
            pp = epool.tile([128, C], F32, tag="ep")
            nc.tensor.matmul(pp, lhsT=WP, rhs=atn, start=True, stop=True)
            otb = esb.tile([128, C], F32, tag="otb")
            nc.vector.tensor_scalar_add(out=otb, in0=pp, scalar1=BP2)
            pf = epool.tile([C, 128], F32, tag="ep")
            nc.tensor.transpose(out=pf, in_=otb, identity=ID)
            nc.vector.tensor_tensor(out=OUT[:, csl], in0=pf, in1=X[:, csl], op=add)
        nc.sync.dma_start(out=y_d[:, nsl], in_=OUT[:, nsl])

    for ch in range(NCH):
        nsl = slice(ch * CHW, (ch + 1) * CHW)
        po = opool.tile([128, CHW], F32, tag="po")
        for gi, (m0, gsz) in enumerate(GROUPS):
            ps = spool.tile([128, 3 * CHW], F32, tag="ps")
            for t in range(gsz):
                m = m0 + t
                nc.tensor.matmul(
                    ps[:, t * CHW:(t + 1) * CHW],
                    lhsT=K[:, m * 128:(m + 1) * 128],
                    rhs=Q[:, nsl],
                    start=True, stop=True)
            pt = ptpool.tile([128, 3 * CHW], BF16, tag="pt")
            nc.scalar.activation(out=pt[:, 0:gsz * CHW], in_=ps[:, 0:gsz * CHW],
                                 func=Exp, bias=ZB, scale=SCALE)
            for t in range(gsz):
                m = m0 + t
                nc.tensor.matmul(
                    po[0:65, :],
                    lhsT=VT1[:, m, :],
                    rhs=pt[:, t * CHW:(t + 1) * CHW],
                    start=(m == 0), stop=(m == MT - 1),
                    skip_group_check=True)
            if gi == 3 and ch > 0:
                epilogue(ch - 1)
        osb = esb.tile([65, CHW], F32, tag="osb")
        nc.vector.tensor_copy(out=osb, in_=po[0:65, :])
        osbs[ch] = osb
    epilogue(NCH - 1)


def build_nc():
    nc = bacc.Bacc("TRN2", target_bir_lowering=False, debug=False)
    shapes = {
        "x": ([C, N], F32), "gamma": ([C, 1], F32), "beta": ([C, 1], F32),
        "p2": ([C, C], F32),
        "wqbd": ([128, 128], BF16), "wkbd": ([128, 128], BF16),
        "wvbd": ([128, 128], BF16), "wpbd": ([128, 128], F32),
        "bq2d": ([C, 1024], F32), "bk2d": ([C, 1024], F32),
        "bv2": ([128, 1], F32), "bp2": ([128, 1], F32),
        "ident": ([128, 128], F32), "identb": ([128, 128], BF16),
    }
    ins = {k: nc.dram_tensor(k, shp, dt, kind="ExternalInput").ap()
           for k, (shp, dt) in shapes.items()}
    y_d = nc.dram_tensor("y", [C, N], F32, kind="ExternalOutput").ap()
    with tile.TileContext(nc) as tc:
        with ExitStack() as ctx:
            attn_body(ctx, tc, ins, y_d)
    nc.compile()
    return nc


def host_params(inputs):
    """Build the small derived parameter arrays shared by all cores."""
    import ml_dtypes
    f = lambda k: np.asarray(inputs[k], np.float32)
    p = {}
    for nm, w, dt in (("wqbd", "Wq", ml_dtypes.bfloat16),
                      ("wkbd", "Wk", ml_dtypes.bfloat16),
                      ("wvbd", "Wv", ml_dtypes.bfloat16),
                      ("wpbd", "Wp", np.float32)):
        W = f(w)
        bd = np.zeros((128, 128), np.float32)
        bd[0:64, 0:64] = W.T
        bd[64:128, 64:128] = W.T
        p[nm] = bd.astype(dt)
    p["gamma"] = f("gn_w").reshape(C, 1)
    p["beta"] = f("gn_b").reshape(C, 1)
    p["bq2d"] = np.broadcast_to(np.tile(f("bq"), 16)[None, :], (C, 1024)).copy()
    p["bk2d"] = np.broadcast_to(np.tile(f("bk"), 16)[None, :], (C, 1024)).copy()
    p["bv2"] = np.tile(f("bv"), 2).reshape(128, 1).astype(np.float32)
    p["bp2"] = np.tile(f("bp"), 2).reshape(128, 1).astype(np.float32)
    p["ident"] = np.eye(128, dtype=np.float32)
    p["identb"] = np.eye(128, dtype=np.float32).astype(ml_dtypes.bfloat16)
    p2 = np.zeros((C, C), np.float32)
    for g in range(C // 2):
        p2[2 * g:2 * g + 2, 2 * g:2 * g + 2] = 0.5
    p["p2"] = p2
    return p


_NC_CACHE = {}


def get_nc():
    if "nc" not in _NC_CACHE:
        _NC_CACHE["nc"] = build_nc()
    return _NC_CACHE["nc"]


def make_in_maps(inputs):
    x = np.asarray(inputs["x"], np.float32)
    B = x.shape[0]
    p = host_params(inputs)
    return [dict(p, x=np.ascontiguousarray(x[b].reshape(C, N))) for b in range(B)]


def kernel(**inputs):
    from concourse.bass_utils import run_bass_kernel_spmd
    x = np.asarray(inputs["x"], np.float32)
    B = x.shape[0]
    nc = get_nc()
    in_maps = make_in_maps(inputs)
    res = run_bass_kernel_spmd(nc, in_maps, core_ids=list(range(B)))
    y = np.stack([res.results[b]["y"].reshape(C, 64, 64) for b in range(B)])
    return y.astype(np.float32)


# revision 6
# speedup vs baseline: 1.6385x; 1.2976x over previous
"""Trainium2 Bass kernel for nn_AttnBlock (B=8, C=64, H=W=64).

Data-parallel: 1 batch per NeuronCore (8 cores). Per core, full
flash-style attention over N=4096 positions with C=64 channels,
never materializing the (N, N) score tensor in HBM.

Per-core pipeline (all on-chip, x kept resident in SBUF):
  1. GroupNorm(32 groups of 2 channels) via bn_stats + pair-combine matmul.
  2. Transpose xn (c, n) -> XT (n, c) bf16 tiles via PE transposes.
  3. Width-axis linear projections q/k/v (bf16) using block-diagonal
     weights: q, k in (c, n) layout; v in (n, c) layout with a ones
     column (row sums of exp(S) fall out of the AV matmul for free).
  4. For each 512-wide chunk of query positions:
       S^T tiles via bf16 matmuls (contraction over c),
       P = exp(S * C^-0.5) on the scalar engine straight out of PSUM,
       attn_out[c, n] (+ row-sum l[n]) accumulated via bf16 AV matmuls.
     The normalize/project/residual epilogue of chunk i is emitted in
     the middle of chunk i+1's main loop so the PE never starves.

Self-contained: hardcodes all shapes; no file reads.
"""

import numpy as np
from contextlib import ExitStack

import concourse.bass as bass
import concourse.bacc as bacc
import concourse.tile as tile
from concourse import mybir

F32 = mybir.dt.float32
BF16 = mybir.dt.bfloat16

C = 64
N = 4096          # H*W
NCH = 8           # n-chunks of 512
CHW = 512         # chunk width
MT = 32           # m-tiles of 128
EPS = 1e-5
SCALE = 1.0 / 8.0  # C ** -0.5

# m-tile groups per exp-activation (3 psum banks per s-tile, double buffered)
GROUPS = [(m0, min(3, MT - m0)) for m0 in range(0, MT, 3)]


def attn_body(ctx: ExitStack, tc: "tile.TileContext", ins: dict, y_d):
    nc = tc.nc
    Exp = mybir.ActivationFunctionType.Exp
    Sqrt = mybir.ActivationFunctionType.Sqrt
    mult = mybir.AluOpType.mult
    add = mybir.AluOpType.add
    sub = mybir.AluOpType.subtract

    persist = ctx.enter_context(tc.tile_pool(name="persist", bufs=1))
    sm = ctx.enter_context(tc.tile_pool(name="sm", bufs=2))

    # ---- persistent SBUF tiles ----
    X = persist.tile([C, N], F32, tag="X")
    XN = persist.tile([C, N], BF16, tag="XN")
    XT = persist.tile([128, 2048], BF16, tag="XT")      # ((h,w), c) chunks
    Q = persist.tile([C, N], BF16, tag="Q")
    K = persist.tile([C, N], BF16, tag="K")
    VT1 = persist.tile([128, MT, 65], BF16, tag="VT1")  # ((H,j), c | 1)
    OUT = persist.tile([C, N], F32, tag="OUT")

    PF = persist.tile([128, 2372], F32, tag="PF")
    PB = persist.tile([128, 512], BF16, tag="PB")
    ZB = persist.tile([128, 1], F32, tag="ZB")
    WP = PF[:, 0:128]
    ID = PF[:, 128:256]
    BQ2D = PF[0:C, 256:1280]
    BK2D = PF[0:C, 1280:2304]
    P2 = PF[0:C, 2304:2368]
    GM = PF[0:C, 2368:2369]
    BT = PF[0:C, 2369:2370]
    BV2 = PF[:, 2370:2371]
    BP2 = PF[:, 2371:2372]
    WQ = PB[:, 0:128]
    WK = PB[:, 128:256]
    WV = PB[:, 256:384]
    IDB = PB[:, 384:512]

    # ---- DMA inputs; bn_stats overlapped with x slices ----
    nc.sync.dma_start(out=PF, in_=ins["pf32"])
    nc.sync.dma_start(out=PB, in_=ins["pb16"])
    nc.vector.memset(ZB, 0.0)
    nc.vector.memset(VT1[:, :, 64:65], 1.0)
    stats = sm.tile([C, 8, 6], F32, tag="stats")
    xg = X.rearrange("p (s f) -> p s f", s=8)
    for i in range(4):
        nc.sync.dma_start(out=X[:, i * 1024:(i + 1) * 1024],
                          in_=ins["x"][:, i * 1024:(i + 1) * 1024])
        for s in (2 * i, 2 * i + 1):
            nc.vector.bn_stats(out=stats[:, s, :], in_=xg[:, s, :])

    # ---- GroupNorm stats ----
    with tc.tile_pool(name="pst", space="PSUM", bufs=2) as pst:
        mv = sm.tile([C, 2], F32, tag="mv")
        nc.vector.bn_aggr(out=mv, in_=stats)

        # st = [mean, E[x^2]] per channel
        st = sm.tile([C, 2], F32, tag="st")
        nc.vector.tensor_copy(out=st[:, 0:1], in_=mv[:, 0:1])
        msq = sm.tile([C, 1], F32, tag="msq")
        nc.vector.tensor_tensor(out=msq, in0=mv[:, 0:1], in1=mv[:, 0:1], op=mult)
        nc.vector.tensor_tensor(out=st[:, 1:2], in0=msq, in1=mv[:, 1:2], op=add)

        # group (pair) averages, broadcast back to both partitions
        mg_ps = pst.tile([C, 2], F32, tag="tp4")
        nc.tensor.matmul(mg_ps, lhsT=P2, rhs=st, start=True, stop=True)

        mu = sm.tile([C, 1], F32, tag="mu")
        nc.vector.tensor_copy(out=mu, in_=mg_ps[:, 0:1])
        musq = sm.tile([C, 1], F32, tag="musq")
        nc.vector.tensor_tensor(out=musq, in0=mu, in1=mu, op=mult)
        ve = sm.tile([C, 1], F32, tag="ve")
        nc.vector.tensor_tensor(out=ve, in0=mg_ps[:, 1:2], in1=musq, op=sub)
        nc.vector.tensor_scalar_add(out=ve, in0=ve, scalar1=EPS)

        sq = sm.tile([C, 1], F32, tag="sq")
        nc.scalar.activation(out=sq, in_=ve, func=Sqrt, bias=ZB[0:C, :], scale=1.0)
        r0 = sm.tile([C, 1], F32, tag="r0")
        nc.vector.reciprocal(out=r0, in_=sq)
        # one Newton step for rsqrt: y1 = r0 * (1.5 - 0.5 * ve * r0^2)
        t0 = sm.tile([C, 1], F32, tag="t0")
        nc.vector.tensor_tensor(out=t0, in0=r0, in1=r0, op=mult)
        nc.vector.tensor_tensor(out=t0, in0=t0, in1=ve, op=mult)
        nc.vector.tensor_scalar(out=t0, in0=t0, scalar1=-0.5, scalar2=1.5,
                                op0=mult, op1=add)
        rstd = sm.tile([C, 1], F32, tag="rstd")
        nc.vector.tensor_tensor(out=rstd, in0=r0, in1=t0, op=mult)

        sc = sm.tile([C, 1], F32, tag="sc")
        nc.vector.tensor_tensor(out=sc, in0=rstd, in1=GM, op=mult)
        t3 = sm.tile([C, 1], F32, tag="t3")
        nc.vector.tensor_tensor(out=t3, in0=mu, in1=sc, op=mult)
        sh = sm.tile([C, 1], F32, tag="sh")
        nc.vector.tensor_tensor(out=sh, in0=BT, in1=t3, op=sub)

        # ---- per 1024-col block: normalize, transpose, project ----
        for blk in range(4):
            bsl = slice(blk * 1024, (blk + 1) * 1024)
            nc.vector.tensor_scalar(out=XN[:, bsl], in0=X[:, bsl],
                                    scalar1=sc, scalar2=sh, op0=mult, op1=add)
            # transpose 8 x (64,128) -> (128,64), batched 4 per psum tile
            for g in range(2):
                tp4 = pst.tile([128, 256], BF16, tag="tp4")
                for t in range(4):
                    i = blk * 8 + g * 4 + t
                    nc.tensor.transpose(out=tp4[:, t * 64:(t + 1) * 64],
                                        in_=XN[:, i * 128:(i + 1) * 128],
                                        identity=IDB[0:C, 0:C])
                i0 = blk * 8 + g * 4
                nc.vector.tensor_copy(out=XT[:, i0 * 64:(i0 + 4) * 64], in_=tp4)
            # q, k projections (8 matmuls -> one (64,1024) psum tile each)
            qp8 = pst.tile([C, 1024], F32, tag="qk")
            for t in range(8):
                i = blk * 8 + t
                nc.tensor.matmul(qp8[:, t * 128:(t + 1) * 128],
                                 lhsT=XT[:, i * C:(i + 1) * C], rhs=WQ,
                                 start=True, stop=True)
            nc.vector.tensor_tensor(out=Q[:, bsl], in0=qp8, in1=BQ2D, op=add)
            kp8 = pst.tile([C, 1024], F32, tag="qk")
            for t in range(8):
                i = blk * 8 + t
                nc.tensor.matmul(kp8[:, t * 128:(t + 1) * 128],
                                 lhsT=XT[:, i * C:(i + 1) * C], rhs=WK,
                                 start=True, stop=True)
            nc.vector.tensor_tensor(out=K[:, bsl], in0=kp8, in1=BK2D, op=add)
            # v projection (4 matmuls per (128,256) psum tile)
            for g in range(2):
                vp4 = pst.tile([128, 256], F32, tag="v4")
                for t in range(4):
                    i = blk * 8 + g * 4 + t
                    nc.tensor.matmul(vp4[:, t * 64:(t + 1) * 64],
                                     lhsT=WV, rhs=XT[:, i * C:(i + 1) * C],
                                     start=True, stop=True)
                i0 = blk * 8 + g * 4
                nc.vector.tensor_scalar_add(
                    out=VT1[:, i0:i0 + 4, 0:C],
                    in0=vp4.rearrange("p (a b) -> p a b", a=4),
                    scalar1=BV2)

    tc.strict_bb_all_engine_barrier()

    # ---- attention ----
    spool = ctx.enter_context(tc.tile_pool(name="spool", space="PSUM", bufs=2))
    opool = ctx.enter_context(tc.tile_pool(name="opool", space="PSUM", bufs=1))
    epool = ctx.enter_context(tc.tile_pool(name="epool", space="PSUM", bufs=1))
    ptpool = ctx.enter_context(tc.tile_pool(name="ptpool", bufs=3))
    esb = ctx.enter_context(tc.tile_pool(name="esb", bufs=2))

    osbs = {}

    def epilogue_steps(ch):
        """Normalize by 1/l, project through Wp, add bias+residual, DMA out.

        Generator: one small PE op (+ its DVE preludes) per step, so steps
        can be interleaved between attention groups as PE gap filler."""
        osb = osbs.pop(ch)
        nsl = slice(ch * CHW, (ch + 1) * CHW)
        for s4 in range(4):
            csl = slice(ch * CHW + s4 * 128, ch * CHW + (s4 + 1) * 128)
            pat = epool.tile([128, 65], F32, tag="ep")
            nc.tensor.transpose(out=pat, in_=osb[:, s4 * 128:(s4 + 1) * 128],
                                identity=ID[0:65, 0:65])
            yield
            rli = esb.tile([128, 1], F32, tag="rli")
            nc.vector.reciprocal(out=rli, in_=pat[:, 64:65])
            atn = esb.tile([128, C], F32, tag="atn")
            nc.vector.tensor_scalar_mul(out=atn, in0=pat[:, 0:C], scalar1=rli)
            pp = epool.tile([128, C], F32, tag="ep")
            nc.tensor.matmul(pp, lhsT=WP, rhs=atn, start=True, stop=True)
            yield
            otb = esb.tile([128, C], F32, tag="otb")
            nc.vector.tensor_scalar_add(out=otb, in0=pp, scalar1=BP2)
            pf = epool.tile([C, 128], F32, tag="ep")
            nc.tensor.transpose(out=pf, in_=otb, identity=ID)
            nc.vector.tensor_tensor(out=OUT[:, csl], in0=pf, in1=X[:, csl], op=add)
            yield
        nc.sync.dma_start(out=y_d[:, nsl], in_=OUT[:, nsl])

    pending = None
    for ch in range(NCH):
        nsl = slice(ch * CHW, (ch + 1) * CHW)
        po = opool.tile([128, CHW], F32, tag="po")
        for gi, (m0, gsz) in enumerate(GROUPS):
            ps = spool.tile([128, 3 * CHW], F32, tag="ps")
            for t in range(gsz):
                m = m0 + t
                nc.tensor.matmul(
                    ps[:, t * CHW:(t + 1) * CHW],
                    lhsT=K[:, m * 128:(m + 1) * 128],
                    rhs=Q[:, nsl],
                    start=True, stop=True)
            pt = ptpool.tile([128, 3 * CHW], BF16, tag="pt")
            nc.scalar.activation(out=pt[:, 0:gsz * CHW], in_=ps[:, 0:gsz * CHW],
                                 func=Exp, bias=ZB, scale=SCALE)
            for t in range(gsz):
                m = m0 + t
                nc.tensor.matmul(
                    po[0:65, :],
                    lhsT=VT1[:, m, :],
                    rhs=pt[:, t * CHW:(t + 1) * CHW],
                    start=(m == 0), stop=(m == MT - 1),
                    skip_group_check=True)
            if pending is not None:
                next(pending, None)
        osb = esb.tile([65, CHW], F32, tag="osb")
        nc.vector.tensor_copy(out=osb, in_=po[0:65, :])
        osbs[ch] = osb
        if pending is not None:
            for _ in pending:
                pass
        pending = epilogue_steps(ch)
    for _ in pending:
        pass


def build_nc():
    nc = bacc.Bacc("TRN2", target_bir_lowering=False, debug=False)
    shapes = {
        "x": ([C, N], F32),
        "pf32": ([128, 2372], F32),
        "pb16": ([128, 512], BF16),
    }
    ins = {k: nc.dram_tensor(k, shp, dt, kind="ExternalInput").ap()
           for k, (shp, dt) in shapes.items()}
    y_d = nc.dram_tensor("y", [C, N], F32, kind="ExternalOutput").ap()
    with tile.TileContext(nc) as tc:
        with ExitStack() as ctx:
            attn_body(ctx, tc, ins, y_d)
    nc.compile()
    return nc


def host_params(inputs):
    """Build the packed parameter arrays shared by all cores."""
    import ml_dtypes
    f = lambda k: np.asarray(inputs[k], np.float32)

    def blockdiag(W):
        bd = np.zeros((128, 128), np.float32)
        bd[0:64, 0:64] = W.T
        bd[64:128, 64:128] = W.T
        return bd

    pf = np.zeros((128, 2372), np.float32)
    pf[:, 0:128] = blockdiag(f("Wp"))
    pf[:, 128:256] = np.eye(128, dtype=np.float32)
    pf[0:C, 256:1280] = np.tile(f("bq"), 16)[None, :]
    pf[0:C, 1280:2304] = np.tile(f("bk"), 16)[None, :]
    p2 = np.zeros((C, C), np.float32)
    for g in range(C // 2):
        p2[2 * g:2 * g + 2, 2 * g:2 * g + 2] = 0.5
    pf[0:C, 2304:2368] = p2
    pf[0:C, 2368] = f("gn_w")
    pf[0:C, 2369] = f("gn_b")
    pf[:, 2370] = np.tile(f("bv"), 2)
    pf[:, 2371] = np.tile(f("bp"), 2)

    pb = np.zeros((128, 512), np.float32)
    pb[:, 0:128] = blockdiag(f("Wq"))
    pb[:, 128:256] = blockdiag(f("Wk"))
    pb[:, 256:384] = blockdiag(f("Wv"))
    pb[:, 384:512] = np.eye(128, dtype=np.float32)
    return {"pf32": pf, "pb16": pb.astype(ml_dtypes.bfloat16)}


_NC_CACHE = {}


def get_nc():
    if "nc" not in _NC_CACHE:
        _NC_CACHE["nc"] = build_nc()
    return _NC_CACHE["nc"]


def make_in_maps(inputs):
    x = np.asarray(inputs["x"], np.float32)
    B = x.shape[0]
    p = host_params(inputs)
    return [dict(p, x=np.ascontiguousarray(x[b].reshape(C, N))) for b in range(B)]


def kernel(**inputs):
    from concourse.bass_utils import run_bass_kernel_spmd
    x = np.asarray(inputs["x"], np.float32)
    B = x.shape[0]
    nc = get_nc()
    in_maps = make_in_maps(inputs)
    res = run_bass_kernel_spmd(nc, in_maps, core_ids=list(range(B)))
    y = np.stack([res.results[b]["y"].reshape(C, 64, 64) for b in range(B)])
    return y.astype(np.float32)


# revision 7
# speedup vs baseline: 1.8967x; 1.1576x over previous
"""Trainium2 Bass kernel for nn_AttnBlock (B=8, C=64, H=W=64).

Data-parallel: 1 batch per NeuronCore (8 cores). Per core, full
flash-style attention over N=4096 positions with C=64 channels,
never materializing the (N, N) score tensor in HBM.

Per-core pipeline (all on-chip, x kept resident in SBUF):
  1. GroupNorm(32 groups of 2 channels) via bn_stats + pair-combine matmul.
  2. Transpose xn (c, n) -> XT (n, c) bf16 tiles via PE transposes.
  3. Width-axis linear projections q/k/v (bf16) using block-diagonal
     weights: q, k in (c, n) layout; v in (n, c) layout with a ones
     column (row sums of exp(S) fall out of the AV matmul for free).
  4. For each 512-wide chunk of query positions:
       S^T tiles via bf16 matmuls (contraction over c),
       P = exp(S * C^-0.5) on the scalar engine straight out of PSUM,
       attn_out[c, n] (+ row-sum l[n]) accumulated via bf16 AV matmuls.
     The normalize/project/residual epilogue of chunk i is emitted in
     the middle of chunk i+1's main loop so the PE never starves.

Self-contained: hardcodes all shapes; no file reads.
"""

import numpy as np
from contextlib import ExitStack

import concourse.bass as bass
import concourse.bacc as bacc
import concourse.tile as tile
from concourse import mybir

F32 = mybir.dt.float32
BF16 = mybir.dt.bfloat16

C = 64
N = 4096          # H*W
NCH = 8           # n-chunks of 512
CHW = 512         # chunk width
MT = 32           # m-tiles of 128
EPS = 1e-5
SCALE = 1.0 / 8.0  # C ** -0.5

# m-tile pairs per exp-activation: the two S^T matmuls of a pair run
# concurrently in the two 64-row halves of the PE array (contraction = 64)
GROUPS = [(m0, 2) for m0 in range(0, MT, 2)]


def attn_body(ctx: ExitStack, tc: "tile.TileContext", ins: dict, y_d):
    nc = tc.nc
    Exp = mybir.ActivationFunctionType.Exp
    Sqrt = mybir.ActivationFunctionType.Sqrt
    mult = mybir.AluOpType.mult
    add = mybir.AluOpType.add
    sub = mybir.AluOpType.subtract

    persist = ctx.enter_context(tc.tile_pool(name="persist", bufs=1))
    sm = ctx.enter_context(tc.tile_pool(name="sm", bufs=2))

    # ---- persistent SBUF tiles ----
    X = persist.tile([C, N], F32, tag="X")
    XN = persist.tile([C, N], BF16, tag="XN")
    XT = persist.tile([128, 2048], BF16, tag="XT")      # ((h,w), c) chunks
    Q = persist.tile([128, N], BF16, tag="Q")
    K = persist.tile([128, N], BF16, tag="K")
    VT1 = persist.tile([128, MT, 65], BF16, tag="VT1")  # ((H,j), c | 1)
    OUT = persist.tile([C, N], F32, tag="OUT")

    PF = persist.tile([128, 2372], F32, tag="PF")
    PB = persist.tile([128, 512], BF16, tag="PB")
    ZB = persist.tile([128, 1], F32, tag="ZB")
    WP = PF[:, 0:128]
    ID = PF[:, 128:256]
    BQ2D = PF[0:C, 256:1280]
    BK2D = PF[0:C, 1280:2304]
    P2 = PF[0:C, 2304:2368]
    GM = PF[0:C, 2368:2369]
    BT = PF[0:C, 2369:2370]
    BV2 = PF[:, 2370:2371]
    BP2 = PF[:, 2371:2372]
    WQ = PB[:, 0:128]
    WK = PB[:, 128:256]
    WV = PB[:, 256:384]
    IDB = PB[:, 384:512]

    # ---- DMA inputs; bn_stats overlapped with x slices ----
    nc.sync.dma_start(out=PF, in_=ins["pf32"])
    nc.sync.dma_start(out=PB, in_=ins["pb16"])
    nc.vector.memset(ZB, 0.0)
    nc.vector.memset(VT1[:, :, 64:65], 1.0)
    stats = sm.tile([C, 8, 6], F32, tag="stats")
    xg = X.rearrange("p (s f) -> p s f", s=8)
    for i in range(4):
        nc.sync.dma_start(out=X[:, i * 1024:(i + 1) * 1024],
                          in_=ins["x"][:, i * 1024:(i + 1) * 1024])
        for s in (2 * i, 2 * i + 1):
            nc.vector.bn_stats(out=stats[:, s, :], in_=xg[:, s, :])

    # ---- GroupNorm stats ----
    with tc.tile_pool(name="pst", space="PSUM", bufs=2) as pst:
        mv = sm.tile([C, 2], F32, tag="mv")
        nc.vector.bn_aggr(out=mv, in_=stats)

        # st = [mean, E[x^2]] per channel
        st = sm.tile([C, 2], F32, tag="st")
        nc.vector.tensor_copy(out=st[:, 0:1], in_=mv[:, 0:1])
        msq = sm.tile([C, 1], F32, tag="msq")
        nc.vector.tensor_tensor(out=msq, in0=mv[:, 0:1], in1=mv[:, 0:1], op=mult)
        nc.vector.tensor_tensor(out=st[:, 1:2], in0=msq, in1=mv[:, 1:2], op=add)

        # group (pair) averages, broadcast back to both partitions
        mg_ps = pst.tile([C, 2], F32, tag="tp4")
        nc.tensor.matmul(mg_ps, lhsT=P2, rhs=st, start=True, stop=True)

        mu = sm.tile([C, 1], F32, tag="mu")
        nc.vector.tensor_copy(out=mu, in_=mg_ps[:, 0:1])
        musq = sm.tile([C, 1], F32, tag="musq")
        nc.vector.tensor_tensor(out=musq, in0=mu, in1=mu, op=mult)
        ve = sm.tile([C, 1], F32, tag="ve")
        nc.vector.tensor_tensor(out=ve, in0=mg_ps[:, 1:2], in1=musq, op=sub)
        nc.vector.tensor_scalar_add(out=ve, in0=ve, scalar1=EPS)

        sq = sm.tile([C, 1], F32, tag="sq")
        nc.scalar.activation(out=sq, in_=ve, func=Sqrt, bias=ZB[0:C, :], scale=1.0)
        r0 = sm.tile([C, 1], F32, tag="r0")
        nc.vector.reciprocal(out=r0, in_=sq)
        # one Newton step for rsqrt: y1 = r0 * (1.5 - 0.5 * ve * r0^2)
        t0 = sm.tile([C, 1], F32, tag="t0")
        nc.vector.tensor_tensor(out=t0, in0=r0, in1=r0, op=mult)
        nc.vector.tensor_tensor(out=t0, in0=t0, in1=ve, op=mult)
        nc.vector.tensor_scalar(out=t0, in0=t0, scalar1=-0.5, scalar2=1.5,
                                op0=mult, op1=add)
        rstd = sm.tile([C, 1], F32, tag="rstd")
        nc.vector.tensor_tensor(out=rstd, in0=r0, in1=t0, op=mult)

        sc = sm.tile([C, 1], F32, tag="sc")
        nc.vector.tensor_tensor(out=sc, in0=rstd, in1=GM, op=mult)
        t3 = sm.tile([C, 1], F32, tag="t3")
        nc.vector.tensor_tensor(out=t3, in0=mu, in1=sc, op=mult)
        sh = sm.tile([C, 1], F32, tag="sh")
        nc.vector.tensor_tensor(out=sh, in0=BT, in1=t3, op=sub)

        # ---- per 1024-col block: normalize, transpose, project ----
        for blk in range(4):
            bsl = slice(blk * 1024, (blk + 1) * 1024)
            nc.vector.tensor_scalar(out=XN[:, bsl], in0=X[:, bsl],
                                    scalar1=sc, scalar2=sh, op0=mult, op1=add)
            # transpose 8 x (64,128) -> (128,64), batched 4 per psum tile
            for g in range(2):
                tp4 = pst.tile([128, 256], BF16, tag="tp4")
                for t in range(4):
                    i = blk * 8 + g * 4 + t
                    nc.tensor.transpose(out=tp4[:, t * 64:(t + 1) * 64],
                                        in_=XN[:, i * 128:(i + 1) * 128],
                                        identity=IDB[0:C, 0:C])
                i0 = blk * 8 + g * 4
                nc.vector.tensor_copy(out=XT[:, i0 * 64:(i0 + 4) * 64], in_=tp4)
            # q, k projections (8 matmuls -> one (64,1024) psum tile each)
            qp8 = pst.tile([C, 1024], F32, tag="qk")
            for t in range(8):
                i = blk * 8 + t
                nc.tensor.matmul(qp8[:, t * 128:(t + 1) * 128],
                                 lhsT=XT[:, i * C:(i + 1) * C], rhs=WQ,
                                 start=True, stop=True)
            nc.vector.tensor_tensor(out=Q[0:C, bsl], in0=qp8, in1=BQ2D, op=add)
            nc.vector.tensor_copy(out=Q[C:128, bsl], in_=Q[0:C, bsl])
            kp8 = pst.tile([C, 1024], F32, tag="qk")
            for t in range(8):
                i = blk * 8 + t
                nc.tensor.matmul(kp8[:, t * 128:(t + 1) * 128],
                                 lhsT=XT[:, i * C:(i + 1) * C], rhs=WK,
                                 start=True, stop=True)
            nc.vector.tensor_tensor(out=K[0:C, bsl], in0=kp8, in1=BK2D, op=add)
            nc.vector.tensor_copy(out=K[C:128, bsl], in_=K[0:C, bsl])
            # v projection (4 matmuls per (128,256) psum tile)
            for g in range(2):
                vp4 = pst.tile([128, 256], F32, tag="v4")
                for t in range(4):
                    i = blk * 8 + g * 4 + t
                    nc.tensor.matmul(vp4[:, t * 64:(t + 1) * 64],
                                     lhsT=WV, rhs=XT[:, i * C:(i + 1) * C],
                                     start=True, stop=True)
                i0 = blk * 8 + g * 4
                nc.vector.tensor_scalar_add(
                    out=VT1[:, i0:i0 + 4, 0:C],
                    in0=vp4.rearrange("p (a b) -> p a b", a=4),
                    scalar1=BV2)

    tc.strict_bb_all_engine_barrier()

    # ---- attention ----
    spool = ctx.enter_context(tc.tile_pool(name="spool", space="PSUM", bufs=3))
    opool = ctx.enter_context(tc.tile_pool(name="opool", space="PSUM", bufs=1))
    epool = ctx.enter_context(tc.tile_pool(name="epool", space="PSUM", bufs=1))
    ptpool = ctx.enter_context(tc.tile_pool(name="ptpool", bufs=3))
    esb = ctx.enter_context(tc.tile_pool(name="esb", bufs=2))

    osbs = {}

    def epilogue_steps(ch):
        """Normalize by 1/l, project through Wp, add bias+residual, DMA out.

        Generator: one small PE op (+ its DVE preludes) per step, so steps
        can be interleaved between attention groups as PE gap filler."""
        osb = osbs.pop(ch)
        nsl = slice(ch * CHW, (ch + 1) * CHW)
        for s4 in range(4):
            csl = slice(ch * CHW + s4 * 128, ch * CHW + (s4 + 1) * 128)
            pat = epool.tile([128, 65], F32, tag="ep")
            nc.tensor.transpose(out=pat, in_=osb[:, s4 * 128:(s4 + 1) * 128],
                                identity=ID[0:65, 0:65])
            yield
            rli = esb.tile([128, 1], F32, tag="rli")
            nc.vector.reciprocal(out=rli, in_=pat[:, 64:65])
            atn = esb.tile([128, C], F32, tag="atn")
            nc.vector.tensor_scalar_mul(out=atn, in0=pat[:, 0:C], scalar1=rli)
            pp = epool.tile([128, C], F32, tag="ep")
            nc.tensor.matmul(pp, lhsT=WP, rhs=atn, start=True, stop=True)
            yield
            otb = esb.tile([128, C], F32, tag="otb")
            nc.vector.tensor_scalar_add(out=otb, in0=pp, scalar1=BP2)
            pf = epool.tile([C, 128], F32, tag="ep")
            nc.tensor.transpose(out=pf, in_=otb, identity=ID)
            nc.vector.tensor_tensor(out=OUT[:, csl], in0=pf, in1=X[:, csl], op=add)
            yield
        nc.sync.dma_start(out=y_d[:, nsl], in_=OUT[:, nsl])

    pending = None
    for ch in range(NCH):
        nsl = slice(ch * CHW, (ch + 1) * CHW)
        po = opool.tile([128, CHW], F32, tag="po")
        for gi, (m0, gsz) in enumerate(GROUPS):
            ps = spool.tile([128, 2 * CHW], F32, tag="ps")
            for t in range(gsz):
                m = m0 + t
                h = t * C  # 0 -> rows 0:64 (tile 0,0), 1 -> rows 64:128 (tile 64,0)
                nc.tensor.matmul(
                    ps[:, t * CHW:(t + 1) * CHW],
                    lhsT=K[h:h + C, m * 128:(m + 1) * 128],
                    rhs=Q[h:h + C, nsl],
                    start=True, stop=True)
            pt = ptpool.tile([128, 2 * CHW], BF16, tag="pt")
            nc.scalar.activation(out=pt[:, 0:gsz * CHW], in_=ps[:, 0:gsz * CHW],
                                 func=Exp, bias=ZB, scale=SCALE)
            for t in range(gsz):
                m = m0 + t
                nc.tensor.matmul(
                    po[0:65, :],
                    lhsT=VT1[:, m, :],
                    rhs=pt[:, t * CHW:(t + 1) * CHW],
                    start=(m == 0), stop=(m == MT - 1),
                    skip_group_check=True)
            if pending is not None:
                next(pending, None)
        osb = esb.tile([65, CHW], F32, tag="osb")
        nc.vector.tensor_copy(out=osb, in_=po[0:65, :])
        osbs[ch] = osb
        if pending is not None:
            for _ in pending:
                pass
        pending = epilogue_steps(ch)
    for _ in pending:
        pass


def build_nc():
    nc = bacc.Bacc("TRN2", target_bir_lowering=False, debug=False)
    shapes = {
        "x": ([C, N], F32),
        "pf32": ([128, 2372], F32),
        "pb16": ([128, 512], BF16),
    }
    ins = {k: nc.dram_tensor(k, shp, dt, kind="ExternalInput").ap()
           for k, (shp, dt) in shapes.items()}
    y_d = nc.dram_tensor("y", [C, N], F32, kind="ExternalOutput").ap()
    with tile.TileContext(nc) as tc:
        with ExitStack() as ctx:
            attn_body(ctx, tc, ins, y_d)
    nc.compile()
    return nc


def host_params(inputs):
    """Build the packed parameter arrays shared by all cores."""
    import ml_dtypes
    f = lambda k: np.asarray(inputs[k], np.float32)

    def blockdiag(W):
        bd = np.zeros((128, 128), np.float32)
        bd[0:64, 0:64] = W.T
        bd[64:128, 64:128] = W.T
        return bd

    pf = np.zeros((128, 2372), np.float32)
    pf[:, 0:128] = blockdiag(f("Wp"))
    pf[:, 128:256] = np.eye(128, dtype=np.float32)
    pf[0:C, 256:1280] = np.tile(f("bq"), 16)[None, :]
    pf[0:C, 1280:2304] = np.tile(f("bk"), 16)[None, :]
    p2 = np.zeros((C, C), np.float32)
    for g in range(C // 2):
        p2[2 * g:2 * g + 2, 2 * g:2 * g + 2] = 0.5
    pf[0:C, 2304:2368] = p2
    pf[0:C, 2368] = f("gn_w")
    pf[0:C, 2369] = f("gn_b")
    pf[:, 2370] = np.tile(f("bv"), 2)
    pf[:, 2371] = np.tile(f("bp"), 2)

    pb = np.zeros((128, 512), np.float32)
    pb[:, 0:128] = blockdiag(f("Wq"))
    pb[:, 128:256] = blockdiag(f("Wk"))
    pb[:, 256:384] = blockdiag(f("Wv"))
    pb[:, 384:512] = np.eye(128, dtype=np.float32)
    return {"pf32": pf, "pb16": pb.astype(ml_dtypes.bfloat16)}


_NC_CACHE = {}


def get_nc():
    if "nc" not in _NC_CACHE:
        _NC_CACHE["nc"] = build_nc()
    return _NC_CACHE["nc"]


def make_in_maps(inputs):
    x = np.asarray(inputs["x"], np.float32)
    B = x.shape[0]
    p = host_params(inputs)
    return [dict(p, x=np.ascontiguousarray(x[b].reshape(C, N))) for b in range(B)]


def kernel(**inputs):
    from concourse.bass_utils import run_bass_kernel_spmd
    x = np.asarray(inputs["x"], np.float32)
    B = x.shape[0]
    nc = get_nc()
    in_maps = make_in_maps(inputs)
    res = run_bass_kernel_spmd(nc, in_maps, core_ids=list(range(B)))
    y = np.stack([res.results[b]["y"].reshape(C, 64, 64) for b in range(B)])
    return y.astype(np.float32)


# revision 8
# speedup vs baseline: 1.9073x; 1.0056x over previous
"""Trainium2 Bass kernel for nn_AttnBlock (B=8, C=64, H=W=64).

Data-parallel: 1 batch per NeuronCore (8 cores). Per core, full
flash-style attention over N=4096 positions with C=64 channels,
never materializing the (N, N) score tensor in HBM.

Per-core pipeline (all on-chip, x kept resident in SBUF):
  1. GroupNorm(32 groups of 2 channels) via bn_stats + pair-combine matmul.
  2. Transpose xn (c, n) -> XT (n, c) bf16 tiles via PE transposes.
  3. Width-axis linear projections q/k/v (bf16) using block-diagonal
     weights: q, k in (c, n) layout; v in (n, c) layout with a ones
     column (row sums of exp(S) fall out of the AV matmul for free).
  4. For each 512-wide chunk of query positions:
       S^T tiles via bf16 matmuls (contraction over c),
       P = exp(S * C^-0.5) on the scalar engine straight out of PSUM,
       attn_out[c, n] (+ row-sum l[n]) accumulated via bf16 AV matmuls.
     The normalize/project/residual epilogue of chunk i is emitted in
     the middle of chunk i+1's main loop so the PE never starves.

Self-contained: hardcodes all shapes; no file reads.
"""

import numpy as np
from contextlib import ExitStack

import concourse.bass as bass
import concourse.bacc as bacc
import concourse.tile as tile
from concourse import mybir

F32 = mybir.dt.float32
BF16 = mybir.dt.bfloat16

C = 64
N = 4096          # H*W
NCH = 8           # n-chunks of 512
CHW = 512         # chunk width
MT = 32           # m-tiles of 128
EPS = 1e-5
SCALE = 1.0 / 8.0  # C ** -0.5

# m-tile groups of 3 per exp-activation: the first two S^T matmuls of a
# group run concurrently in the two 64-row halves of the PE array
# (contraction = 64); the third uses the low half.
GROUPS = [(m0, min(3, MT - m0)) for m0 in range(0, MT, 3)]


def attn_body(ctx: ExitStack, tc: "tile.TileContext", ins: dict, y_d):
    nc = tc.nc
    Exp = mybir.ActivationFunctionType.Exp
    Sqrt = mybir.ActivationFunctionType.Sqrt
    mult = mybir.AluOpType.mult
    add = mybir.AluOpType.add
    sub = mybir.AluOpType.subtract

    persist = ctx.enter_context(tc.tile_pool(name="persist", bufs=1))
    sm = ctx.enter_context(tc.tile_pool(name="sm", bufs=2))

    # ---- persistent SBUF tiles ----
    X = persist.tile([C, N], F32, tag="X")
    XN = persist.tile([C, N], BF16, tag="XN")
    XT = persist.tile([128, 2048], BF16, tag="XT")      # ((h,w), c) chunks
    Q = persist.tile([128, N], BF16, tag="Q")
    K = persist.tile([128, N], BF16, tag="K")
    VT1 = persist.tile([128, MT, 65], BF16, tag="VT1")  # ((H,j), c | 1)
    OUT = persist.tile([C, N], F32, tag="OUT")

    PF = persist.tile([128, 2372], F32, tag="PF")
    PB = persist.tile([128, 512], BF16, tag="PB")
    ZB = persist.tile([128, 1], F32, tag="ZB")
    WP = PF[:, 0:128]
    ID = PF[:, 128:256]
    BQ2D = PF[0:C, 256:1280]
    BK2D = PF[0:C, 1280:2304]
    P2 = PF[0:C, 2304:2368]
    GM = PF[0:C, 2368:2369]
    BT = PF[0:C, 2369:2370]
    BV2 = PF[:, 2370:2371]
    BP2 = PF[:, 2371:2372]
    WQ = PB[:, 0:128]
    WK = PB[:, 128:256]
    WV = PB[:, 256:384]
    IDB = PB[:, 384:512]

    # ---- DMA inputs; bn_stats overlapped with x slices ----
    nc.sync.dma_start(out=PF, in_=ins["pf32"])
    nc.sync.dma_start(out=PB, in_=ins["pb16"])
    nc.vector.memset(ZB, 0.0)
    nc.vector.memset(VT1[:, :, 64:65], 1.0)
    stats = sm.tile([C, 8, 6], F32, tag="stats")
    xg = X.rearrange("p (s f) -> p s f", s=8)
    for i in range(4):
        nc.sync.dma_start(out=X[:, i * 1024:(i + 1) * 1024],
                          in_=ins["x"][:, i * 1024:(i + 1) * 1024])
        for s in (2 * i, 2 * i + 1):
            nc.vector.bn_stats(out=stats[:, s, :], in_=xg[:, s, :])

    # ---- GroupNorm stats ----
    with tc.tile_pool(name="pst", space="PSUM", bufs=2) as pst:
        mv = sm.tile([C, 2], F32, tag="mv")
        nc.vector.bn_aggr(out=mv, in_=stats)

        # st = [mean, E[x^2]] per channel
        st = sm.tile([C, 2], F32, tag="st")
        nc.vector.tensor_copy(out=st[:, 0:1], in_=mv[:, 0:1])
        msq = sm.tile([C, 1], F32, tag="msq")
        nc.vector.tensor_tensor(out=msq, in0=mv[:, 0:1], in1=mv[:, 0:1], op=mult)
        nc.vector.tensor_tensor(out=st[:, 1:2], in0=msq, in1=mv[:, 1:2], op=add)

        # group (pair) averages, broadcast back to both partitions
        mg_ps = pst.tile([C, 2], F32, tag="tp4")
        nc.tensor.matmul(mg_ps, lhsT=P2, rhs=st, start=True, stop=True)

        mu = sm.tile([C, 1], F32, tag="mu")
        nc.vector.tensor_copy(out=mu, in_=mg_ps[:, 0:1])
        musq = sm.tile([C, 1], F32, tag="musq")
        nc.vector.tensor_tensor(out=musq, in0=mu, in1=mu, op=mult)
        ve = sm.tile([C, 1], F32, tag="ve")
        nc.vector.tensor_tensor(out=ve, in0=mg_ps[:, 1:2], in1=musq, op=sub)
        nc.vector.tensor_scalar_add(out=ve, in0=ve, scalar1=EPS)

        sq = sm.tile([C, 1], F32, tag="sq")
        nc.scalar.activation(out=sq, in_=ve, func=Sqrt, bias=ZB[0:C, :], scale=1.0)
        r0 = sm.tile([C, 1], F32, tag="r0")
        nc.vector.reciprocal(out=r0, in_=sq)
        # one Newton step for rsqrt: y1 = r0 * (1.5 - 0.5 * ve * r0^2)
        t0 = sm.tile([C, 1], F32, tag="t0")
        nc.vector.tensor_tensor(out=t0, in0=r0, in1=r0, op=mult)
        nc.vector.tensor_tensor(out=t0, in0=t0, in1=ve, op=mult)
        nc.vector.tensor_scalar(out=t0, in0=t0, scalar1=-0.5, scalar2=1.5,
                                op0=mult, op1=add)
        rstd = sm.tile([C, 1], F32, tag="rstd")
        nc.vector.tensor_tensor(out=rstd, in0=r0, in1=t0, op=mult)

        sc = sm.tile([C, 1], F32, tag="sc")
        nc.vector.tensor_tensor(out=sc, in0=rstd, in1=GM, op=mult)
        t3 = sm.tile([C, 1], F32, tag="t3")
        nc.vector.tensor_tensor(out=t3, in0=mu, in1=sc, op=mult)
        sh = sm.tile([C, 1], F32, tag="sh")
        nc.vector.tensor_tensor(out=sh, in0=BT, in1=t3, op=sub)

        # ---- per 1024-col block: normalize, transpose, project ----
        for blk in range(4):
            bsl = slice(blk * 1024, (blk + 1) * 1024)
            nc.vector.tensor_scalar(out=XN[:, bsl], in0=X[:, bsl],
                                    scalar1=sc, scalar2=sh, op0=mult, op1=add)
            # transpose 8 x (64,128) -> (128,64), batched 4 per psum tile
            for g in range(2):
                tp4 = pst.tile([128, 256], BF16, tag="tp4")
                for t in range(4):
                    i = blk * 8 + g * 4 + t
                    nc.tensor.transpose(out=tp4[:, t * 64:(t + 1) * 64],
                                        in_=XN[:, i * 128:(i + 1) * 128],
                                        identity=IDB[0:C, 0:C])
                i0 = blk * 8 + g * 4
                nc.vector.tensor_copy(out=XT[:, i0 * 64:(i0 + 4) * 64], in_=tp4)
            # q, k projections (8 matmuls -> one (64,1024) psum tile each)
            qp8 = pst.tile([C, 1024], F32, tag="qk")
            for t in range(8):
                i = blk * 8 + t
                nc.tensor.matmul(qp8[:, t * 128:(t + 1) * 128],
                                 lhsT=XT[:, i * C:(i + 1) * C], rhs=WQ,
                                 start=True, stop=True)
            nc.vector.tensor_tensor(out=Q[0:C, bsl], in0=qp8, in1=BQ2D, op=add)
            nc.vector.tensor_copy(out=Q[C:128, bsl], in_=Q[0:C, bsl])
            kp8 = pst.tile([C, 1024], F32, tag="qk")
            for t in range(8):
                i = blk * 8 + t
                nc.tensor.matmul(kp8[:, t * 128:(t + 1) * 128],
                                 lhsT=XT[:, i * C:(i + 1) * C], rhs=WK,
                                 start=True, stop=True)
            nc.vector.tensor_tensor(out=K[0:C, bsl], in0=kp8, in1=BK2D, op=add)
            nc.vector.tensor_copy(out=K[C:128, bsl], in_=K[0:C, bsl])
            # v projection (4 matmuls per (128,256) psum tile)
            for g in range(2):
                vp4 = pst.tile([128, 256], F32, tag="v4")
                for t in range(4):
                    i = blk * 8 + g * 4 + t
                    nc.tensor.matmul(vp4[:, t * 64:(t + 1) * 64],
                                     lhsT=WV, rhs=XT[:, i * C:(i + 1) * C],
                                     start=True, stop=True)
                i0 = blk * 8 + g * 4
                nc.vector.tensor_scalar_add(
                    out=VT1[:, i0:i0 + 4, 0:C],
                    in0=vp4.rearrange("p (a b) -> p a b", a=4),
                    scalar1=BV2)

    tc.strict_bb_all_engine_barrier()

    # ---- attention ----
    spool = ctx.enter_context(tc.tile_pool(name="spool", space="PSUM", bufs=2))
    opool = ctx.enter_context(tc.tile_pool(name="opool", space="PSUM", bufs=1))
    epool = ctx.enter_context(tc.tile_pool(name="epool", space="PSUM", bufs=1))
    ptpool = ctx.enter_context(tc.tile_pool(name="ptpool", bufs=3))
    esb = ctx.enter_context(tc.tile_pool(name="esb", bufs=2))

    osbs = {}

    def epilogue_steps(ch):
        """Normalize by 1/l, project through Wp, add bias+residual, DMA out.

        Generator: one small PE op (+ its DVE preludes) per step, so steps
        can be interleaved between attention groups as PE gap filler."""
        osb = osbs.pop(ch)
        nsl = slice(ch * CHW, (ch + 1) * CHW)
        for s4 in range(4):
            csl = slice(ch * CHW + s4 * 128, ch * CHW + (s4 + 1) * 128)
            pat = epool.tile([128, 65], F32, tag="ep")
            nc.tensor.transpose(out=pat, in_=osb[:, s4 * 128:(s4 + 1) * 128],
                                identity=ID[0:65, 0:65])
            yield
            rli = esb.tile([128, 1], F32, tag="rli")
            nc.vector.reciprocal(out=rli, in_=pat[:, 64:65])
            atn = esb.tile([128, C], F32, tag="atn")
            nc.vector.tensor_scalar_mul(out=atn, in0=pat[:, 0:C], scalar1=rli)
            pp = epool.tile([128, C], F32, tag="ep")
            nc.tensor.matmul(pp, lhsT=WP, rhs=atn, start=True, stop=True)
            yield
            otb = esb.tile([128, C], F32, tag="otb")
            nc.vector.tensor_scalar_add(out=otb, in0=pp, scalar1=BP2)
            pf = epool.tile([C, 128], F32, tag="ep")
            nc.tensor.transpose(out=pf, in_=otb, identity=ID)
            nc.vector.tensor_tensor(out=OUT[:, csl], in0=pf, in1=X[:, csl], op=add)
            yield
        nc.sync.dma_start(out=y_d[:, nsl], in_=OUT[:, nsl])

    # PE warm-up burst: ~3.4us of back-to-back matmuls so the HAM clock
    # gate reaches 8/8 before the attention pipeline starts
    wps = spool.tile([128, 3 * CHW], F32, tag="ps")
    for _ in range(8):
        nc.tensor.matmul(wps[:, 0:CHW], lhsT=K[0:C, 0:128], rhs=Q[0:C, 0:CHW],
                         start=True, stop=True)

    pending = None
    for ch in range(NCH):
        nsl = slice(ch * CHW, (ch + 1) * CHW)
        po = opool.tile([128, CHW], F32, tag="po")
        for gi, (m0, gsz) in enumerate(GROUPS):
            ps = spool.tile([128, 3 * CHW], F32, tag="ps")
            for t in range(gsz):
                m = m0 + t
                h = (t % 2) * C  # alternate the two 64-row PE halves
                nc.tensor.matmul(
                    ps[:, t * CHW:(t + 1) * CHW],
                    lhsT=K[h:h + C, m * 128:(m + 1) * 128],
                    rhs=Q[h:h + C, nsl],
                    start=True, stop=True)
            pt = ptpool.tile([128, 3 * CHW], BF16, tag="pt")
            nc.scalar.activation(out=pt[:, 0:gsz * CHW], in_=ps[:, 0:gsz * CHW],
                                 func=Exp, bias=ZB, scale=SCALE)
            for t in range(gsz):
                m = m0 + t
                nc.tensor.matmul(
                    po[0:65, :],
                    lhsT=VT1[:, m, :],
                    rhs=pt[:, t * CHW:(t + 1) * CHW],
                    start=(m == 0), stop=(m == MT - 1),
                    skip_group_check=True)
            if pending is not None:
                next(pending, None)
        osb = esb.tile([65, CHW], F32, tag="osb")
        nc.vector.tensor_copy(out=osb, in_=po[0:65, :])
        osbs[ch] = osb
        if pending is not None:
            for _ in pending:
                pass
        pending = epilogue_steps(ch) if ch < NCH - 1 else None
    # final chunk's epilogue: attention is done, so psum is free — run the
    # four subs in parallel on rotating spool slots instead of one epool slot
    osb = osbs.pop(NCH - 1)
    nsl = slice((NCH - 1) * CHW, NCH * CHW)
    pats, atns, pps, otbs = [], [], [], []
    for s4 in range(4):
        pat = spool.tile([128, 3 * CHW], F32, tag="ps")
        nc.tensor.transpose(out=pat[:, 0:65], in_=osb[:, s4 * 128:(s4 + 1) * 128],
                            identity=ID[0:65, 0:65])
        pats.append(pat)
    for s4 in range(4):
        rli = esb.tile([128, 1], F32, tag=f"rlif{s4}")
        nc.vector.reciprocal(out=rli, in_=pats[s4][:, 64:65])
        atn = esb.tile([128, C], F32, tag=f"atnf{s4}")
        nc.vector.tensor_scalar_mul(out=atn, in0=pats[s4][:, 0:C], scalar1=rli)
        atns.append(atn)
    for s4 in range(4):
        pp = spool.tile([128, 3 * CHW], F32, tag="ps")
        nc.tensor.matmul(pp[:, 0:C], lhsT=WP, rhs=atns[s4], start=True, stop=True)
        pps.append(pp)
    for s4 in range(4):
        otb = esb.tile([128, C], F32, tag=f"otbf{s4}")
        nc.vector.tensor_scalar_add(out=otb, in0=pps[s4][:, 0:C], scalar1=BP2)
        otbs.append(otb)
    for s4 in range(4):
        csl = slice((NCH - 1) * CHW + s4 * 128, (NCH - 1) * CHW + (s4 + 1) * 128)
        pf = spool.tile([128, 3 * CHW], F32, tag="ps")
        nc.tensor.transpose(out=pf[0:C, 0:128], in_=otbs[s4], identity=ID)
        nc.vector.tensor_tensor(out=OUT[:, csl], in0=pf[0:C, 0:128],
                                in1=X[:, csl], op=add)
    nc.sync.dma_start(out=y_d[:, nsl], in_=OUT[:, nsl])


def build_nc():
    nc = bacc.Bacc("TRN2", target_bir_lowering=False, debug=False)
    shapes = {
        "x": ([C, N], F32),
        "pf32": ([128, 2372], F32),
        "pb16": ([128, 512], BF16),
    }
    ins = {k: nc.dram_tensor(k, shp, dt, kind="ExternalInput").ap()
           for k, (shp, dt) in shapes.items()}
    y_d = nc.dram_tensor("y", [C, N], F32, kind="ExternalOutput").ap()
    with tile.TileContext(nc) as tc:
        with ExitStack() as ctx:
            attn_body(ctx, tc, ins, y_d)
    nc.compile()
    return nc


def host_params(inputs):
    """Build the packed parameter arrays shared by all cores."""
    import ml_dtypes
    f = lambda k: np.asarray(inputs[k], np.float32)

    def blockdiag(W):
        bd = np.zeros((128, 128), np.float32)
        bd[0:64, 0:64] = W.T
        bd[64:128, 64:128] = W.T
        return bd

    pf = np.zeros((128, 2372), np.float32)
    pf[:, 0:128] = blockdiag(f("Wp"))
    pf[:, 128:256] = np.eye(128, dtype=np.float32)
    pf[0:C, 256:1280] = np.tile(f("bq"), 16)[None, :]
    pf[0:C, 1280:2304] = np.tile(f("bk"), 16)[None, :]
    p2 = np.zeros((C, C), np.float32)
    for g in range(C // 2):
        p2[2 * g:2 * g + 2, 2 * g:2 * g + 2] = 0.5
    pf[0:C, 2304:2368] = p2
    pf[0:C, 2368] = f("gn_w")
    pf[0:C, 2369] = f("gn_b")
    pf[:, 2370] = np.tile(f("bv"), 2)
    pf[:, 2371] = np.tile(f("bp"), 2)

    pb = np.zeros((128, 512), np.float32)
    pb[:, 0:128] = blockdiag(f("Wq"))
    pb[:, 128:256] = blockdiag(f("Wk"))
    pb[:, 256:384] = blockdiag(f("Wv"))
    pb[:, 384:512] = np.eye(128, dtype=np.float32)
    return {"pf32": pf, "pb16": pb.astype(ml_dtypes.bfloat16)}


_NC_CACHE = {}


def get_nc():
    if "nc" not in _NC_CACHE:
        _NC_CACHE["nc"] = build_nc()
    return _NC_CACHE["nc"]


def make_in_maps(inputs):
    x = np.asarray(inputs["x"], np.float32)
    B = x.shape[0]
    p = host_params(inputs)
    return [dict(p, x=np.ascontiguousarray(x[b].reshape(C, N))) for b in range(B)]


def kernel(**inputs):
    from concourse.bass_utils import run_bass_kernel_spmd
    x = np.asarray(inputs["x"], np.float32)
    B = x.shape[0]
    nc = get_nc()
    in_maps = make_in_maps(inputs)
    res = run_bass_kernel_spmd(nc, in_maps, core_ids=list(range(B)))
    y = np.stack([res.results[b]["y"].reshape(C, 64, 64) for b in range(B)])
    return y.astype(np.float32)
